# revision 3
# baseline (speedup 1.0000x reference)
"""Bass/Trainium2 kernel v3 for nn_KernelAMController (retrieval_knn).

Math: out(b,:) = -sum_g w_eff(b,g)*adj[tb(b),g,:] / (sum_g w_eff(b,g) + eps)
with w_eff(b,g) = exp(-2*||x_b - p_g||^2) * (counts[tb(b),g] > 0).

Design:
  * Spatial pruning: samples sorted by x0 on the host into 64 groups of
    512; each group computes over only NSEL=5 grid chunks of 128 points
    (those within RCUT of its x0 range) instead of all 20 (dropped
    Gaussian mass ~1e-3 relative). The host gathers per-group chunk
    operands so the device program stays static.
  * mm1 (K padded to KPAD): W^T(g,b) = exp(Pa_g^T @ Xa) -- the split-bf16
    augmented quadratic form gives the exponent directly. K is padded with
    zero rows because K=15 matmuls never trip the PE HAM activity monitor
    (the PE stays at 1.2 GHz); padded-K streams warm at 2.4 GHz.
  * mm2: py(m,b) += Ct_c(g,m)^T @ W_c^T(g,b); m = 64 columns
    [-mask*adj_x | -mask*adj_y | mask | pad] per time bin.
  * Epilogue on device is two cheap steps: r3 = py * onehot(tb) (DVE) and
    pr(4,b) = Bn^T @ r3 (one matmul summing each 20-bin segment). The
    final elementwise divide on (B,3) happens on the host after the
    gather, where it is trivial.
  * All inputs land in SBUF via one contiguous DMA per tensor (host
    pre-lays them out partition-major); junk warmup matmuls during the
    preload absorb the PE HAM ramp.
"""
import numpy as np
import ml_dtypes

import concourse.bass as bass
import concourse.tile as tile
from concourse import mybir, bacc
from concourse.bass_utils import run_bass_kernel_spmd

F32 = mybir.dt.float32
BF16 = mybir.dt.bfloat16
BF16_NP = ml_dtypes.bfloat16

B = 32768
G = 2500
GRID_SIZE = 50
NCHUNK = 20
NBINS = 20
NCORES = 8
BC = B // NCORES       # 4096 samples per core
BG = 512               # samples per group
NGRP = BC // BG        # 8 groups per core
NSEL = 5               # chunks kept per group
RCUT = 1.8             # x0 pruning radius
EPS = 1e-10
KPAD = 128             # contraction rows for mm1 (>=64; HAM warmth)
NWARM = 6              # junk warmup matmuls

_CACHE = {}


def _build_nc():
    nc = bacc.Bacc("TRN2", target_bir_lowering=False)
    xa_d = nc.dram_tensor("xa", [KPAD, NGRP * BG], BF16, kind="ExternalInput")
    pa_d = nc.dram_tensor("pa", [KPAD, NGRP * NSEL * 128], BF16,
                          kind="ExternalInput")
    ct_d = nc.dram_tensor("ct", [128, NGRP * NSEL * 64], BF16,
                          kind="ExternalInput")
    o3_d = nc.dram_tensor("o3", [64, NGRP * BG], BF16, kind="ExternalInput")
    bn_d = nc.dram_tensor("bn", [64, 4], BF16, kind="ExternalInput")
    o_d = nc.dram_tensor("o", [NGRP, 4, BG], F32, kind="ExternalOutput")

    with tile.TileContext(nc) as tc:
        with (
            tc.tile_pool(name="consts", bufs=1) as consts,
            tc.tile_pool(name="wt", bufs=3) as wtp,
            tc.tile_pool(name="ep", bufs=2) as ep,
            tc.tile_pool(name="pwa", bufs=1, space="PSUM") as pwa,
            tc.tile_pool(name="pwb", bufs=1, space="PSUM") as pwb,
            tc.tile_pool(name="py", bufs=1, space="PSUM") as pyp,
            tc.tile_pool(name="pr", bufs=1, space="PSUM") as prp,
        ):
            dum = consts.tile([128, BG], BF16)
            nc.vector.memset(dum[:], 0.5)
            pa_all = consts.tile([KPAD, NGRP, NSEL * 128], BF16)
            xa_all = consts.tile([KPAD, NGRP, BG], BF16)
            ct_all = consts.tile([128, NGRP, NSEL * 64], BF16)
            o3_all = consts.tile([64, NGRP, BG], BF16)
            bn_sb = consts.tile([64, 4], BF16)
            nc.sync.dma_start(out=bn_sb[:], in_=bn_d[:])
            CW, XW, OW = NSEL * 128, BG, BG
            for g0, g1 in ((0, 1), (1, 2), (2, 4), (4, 6), (6, 8)):
                nc.sync.dma_start(out=pa_all[:, g0:g1, :],
                                  in_=pa_d[:, g0 * CW:g1 * CW])
                nc.sync.dma_start(out=xa_all[:, g0:g1, :],
                                  in_=xa_d[:, g0 * XW:g1 * XW])
                nc.sync.dma_start(out=ct_all[:, g0:g1, :],
                                  in_=ct_d[:, g0 * NSEL * 64:g1 * NSEL * 64])
                nc.sync.dma_start(out=o3_all[:, g0:g1, :],
                                  in_=o3_d[:, g0 * OW:g1 * OW])

            def py_tile():
                return pyp.tile([64, BG], F32, tag="py", name="py")

            # warm the PE HAM while the preload DMAs land
            for i in range(NWARM):
                wu = py_tile()
                nc.tensor.matmul(wu[:], lhsT=dum[:, 0:64], rhs=dum[:],
                                 start=True, stop=True)

            def tail(state):
                (wt_a, wt_b), g = state
                py = py_tile()
                for c in range(NSEL):
                    nc.tensor.matmul(
                        py[:], lhsT=ct_all[:, g, c * 64:(c + 1) * 64],
                        rhs=(wt_a[:, c, :] if c < 2 else wt_b[:, c - 2, :]),
                        start=(c == 0), stop=(c == NSEL - 1))
                r3 = ep.tile([64, BG], BF16, tag="r3")
                nc.vector.tensor_mul(r3[:], py[:], o3_all[:, g])
                pr = prp.tile([4, BG], F32)
                nc.tensor.matmul(pr[:], lhsT=bn_sb[:], rhs=r3[:],
                                 start=True, stop=True)
                ot = ep.tile([4, BG], F32, tag="ot")
                nc.vector.tensor_copy(ot[:], pr[:])
                nc.sync.dma_start(out=o_d[g], in_=ot[:])

            states = []
            for g in range(NGRP):
                pw_a = pwa.tile([128, 2, BG], F32)
                for j in range(2):
                    nc.tensor.matmul(
                        pw_a[:, j, :],
                        lhsT=pa_all[:, g, j * 128:(j + 1) * 128],
                        rhs=xa_all[:, g], start=True, stop=True)
                wt_a = wtp.tile([128, 2, BG], BF16, tag="a")
                nc.scalar.activation(wt_a[:], pw_a[:],
                                     mybir.ActivationFunctionType.Exp)
                pw_b = pwb.tile([128, 3, BG], F32)
                for j in range(3):
                    c = 2 + j
                    nc.tensor.matmul(
                        pw_b[:, j, :],
                        lhsT=pa_all[:, g, c * 128:(c + 1) * 128],
                        rhs=xa_all[:, g], start=True, stop=True)
                wt_b = wtp.tile([128, 3, BG], BF16, tag="b")
                nc.scalar.activation(wt_b[:], pw_b[:],
                                     mybir.ActivationFunctionType.Exp)
                states.append(((wt_a, wt_b), g))
                if len(states) > 2:
                    tail(states.pop(0))
            for st in states:
                tail(st)
    nc.compile()
    return nc


_LIN = np.linspace(-8.0, 8.0, GRID_SIZE).astype(np.float32)
_CHUNK_LO = np.array([_LIN[(128 * c) // GRID_SIZE] for c in range(NCHUNK)])
_CHUNK_HI = np.array([_LIN[min((128 * c + 127) // GRID_SIZE, GRID_SIZE - 1)]
                      for c in range(NCHUNK)])


def _host_prep(t, x, grid_points, grid_adjoints, t_edges, grid_counts):
    t = np.asarray(t, np.float32).reshape(B)
    x = np.asarray(x, np.float32)
    gp = np.asarray(grid_points, np.float32)
    adj = np.asarray(grid_adjoints, np.float32)
    te = np.asarray(t_edges, np.float32)
    cnt = np.asarray(grid_counts)

    # global x0 sort -> 64 groups of 512 with narrow x0 bands
    perm = np.argsort(x[:, 0], kind='stable')
    xs = x[perm]
    ts = t[perm]

    # time-bin index (searchsorted-left semantics, clamped)
    tb = np.clip(np.searchsorted(te[1:-1], ts, side='left'), 0, NBINS - 1)

    # augmented X (KPAD, B): split-bf16 [xh; sqh; 1 | xl; sql; 0 | xh; sqh; 1]
    sq = xs * xs
    xh = xs.astype(BF16_NP)
    xl = (xs - xh.astype(np.float32)).astype(BF16_NP)
    sqh = sq.astype(BF16_NP)
    sql = (sq - sqh.astype(np.float32)).astype(BF16_NP)
    xa_full = np.zeros((KPAD, B), BF16_NP)
    for base, (c0, c1, one) in zip((0, 5, 10),
                                   ((xh, sqh, 1.0), (xl, sql, 0.0),
                                    (xh, sqh, 1.0))):
        xa_full[base + 0] = c0[:, 0]
        xa_full[base + 1] = c0[:, 1]
        xa_full[base + 2] = c1[:, 0]
        xa_full[base + 3] = c1[:, 1]
        xa_full[base + 4] = np.float32(one)

    # grid-side split-bf16 operand (15, GP)
    GP = NCHUNK * 128
    p5 = np.zeros((5, GP), np.float32)
    p5[0, :G] = 4.0 * gp[:, 0]
    p5[1, :G] = 4.0 * gp[:, 1]
    p5[2, :G] = -2.0
    p5[3, :G] = -2.0
    p5[4, :G] = -2.0 * (gp[:, 0] ** 2 + gp[:, 1] ** 2)
    p5[4, G:] = -1e30
    ph = p5.astype(BF16_NP)
    pl = (p5 - ph.astype(np.float32)).astype(BF16_NP)
    pa_full = np.zeros((KPAD, GP), BF16_NP)
    pa_full[0:5] = ph
    pa_full[5:10] = ph
    pa_full[10:15] = pl

    # ct (GP, 64): [-mask*adj_x | -mask*adj_y | mask | 0pad] per bin
    mask = (cnt > 0).astype(np.float32)                 # (20, G)
    ct_full = np.zeros((GP, 64), np.float32)
    ct_full[:G, 0:20] = -(mask * adj[:, :, 0]).T
    ct_full[:G, 20:40] = -(mask * adj[:, :, 1]).T
    ct_full[:G, 40:60] = mask.T
    ct_full = ct_full.astype(BF16_NP)

    # one-hot (64, B): rows tb, 20+tb, 40+tb are 1
    o3_full = np.zeros((64, B), BF16_NP)
    ar = np.arange(B)
    for dcol in range(3):
        o3_full[dcol * 20 + tb, ar] = np.float32(1.0)

    bn = np.zeros((64, 4), BF16_NP)
    for dcol in range(3):
        bn[dcol * 20:(dcol + 1) * 20, dcol] = np.float32(1.0)

    ngrp_total = B // BG
    sel_all = []
    for i in range(ngrp_total):
        seg = xs[i * BG:(i + 1) * BG, 0]
        a, b = seg.min(), seg.max()
        sel = [c for c in range(NCHUNK)
               if _CHUNK_HI[c] >= a - RCUT and _CHUNK_LO[c] <= b + RCUT]
        if not sel:
            sel = [-1]                   # no valid chunk: all-zero ct
        if len(sel) > NSEL:
            mid = 0.5 * (a + b)
            sel = sorted(sel,
                         key=lambda c: abs(0.5 * (_CHUNK_LO[c]
                                                  + _CHUNK_HI[c]) - mid))
            sel = sorted(sel[:NSEL])
        while len(sel) < NSEL:
            sel.append(sel[-1])          # duplicate pad; ct left zero
        sel_all.append(sel)

    in_maps = []
    for i in range(NCORES):
        xa_c = np.zeros((KPAD, NGRP, BG), BF16_NP)
        pa_c = np.zeros((KPAD, NGRP, NSEL * 128), BF16_NP)
        ct_c = np.zeros((128, NGRP, NSEL * 64), BF16_NP)
        o3_c = np.zeros((64, NGRP, BG), BF16_NP)
        for gl in range(NGRP):
            gi = i * NGRP + gl
            s0 = gi * BG
            xa_c[:, gl, :] = xa_full[:, s0:s0 + BG]
            o3_c[:, gl, :] = o3_full[:, s0:s0 + BG]
            sel = sel_all[gi]
            seen = set()
            for k, c in enumerate(sel):
                if c < 0:
                    pa_c[4, gl, k * 128:(k + 1) * 128] = np.float32(-1e30)
                    continue
                pa_c[:, gl, k * 128:(k + 1) * 128] = \
                    pa_full[:, c * 128:(c + 1) * 128]
                if c not in seen:
                    ct_c[:, gl, k * 64:(k + 1) * 64] = \
                        ct_full[c * 128:(c + 1) * 128, :]
                    seen.add(c)
        in_maps.append({
            "xa": np.ascontiguousarray(xa_c.reshape(KPAD, NGRP * BG)),
            "pa": np.ascontiguousarray(pa_c.reshape(KPAD, NGRP * NSEL * 128)),
            "ct": np.ascontiguousarray(ct_c.reshape(128, NGRP * NSEL * 64)),
            "o3": np.ascontiguousarray(o3_c.reshape(64, NGRP * BG)),
            "bn": bn,
        })
    return in_maps, perm


def kernel(t, x, grid_points, grid_adjoints, t_edges, grid_counts,
           trace=False, tmpdir=None):
    if "nc" not in _CACHE:
        _CACHE["nc"] = _build_nc()
    nc = _CACHE["nc"]
    in_maps, perm = _host_prep(t, x, grid_points, grid_adjoints, t_edges,
                               grid_counts)
    res = run_bass_kernel_spmd(nc, in_maps, core_ids=list(range(NCORES)),
                               trace=trace, tmpdir=tmpdir)
    _CACHE["last_result"] = res
    pys = np.concatenate([res.results[i]["o"].reshape(NGRP, 4, BG)
                          for i in range(NCORES)], axis=0)   # (64, 4, 512)
    pys = pys.transpose(0, 2, 1).reshape(B, 4)
    den = pys[:, 2] + np.float32(EPS)
    out_sorted = pys[:, 0:2] / den[:, None]
    out = np.empty((B, 2), np.float32)
    out[perm] = out_sorted.astype(np.float32)
    return out
